# revision 9
# baseline (speedup 1.0000x reference)
"""Trainium2 Bass kernel for single-head attention + output projection + residual.

Math per batch element b (N=2048, D=512, U=128):
    Q = x @ W_q; K = x @ W_k; V = x @ W_v
    S = Q @ K.T / sqrt(U); A = softmax(S, axis=-1)
    out = (A @ V) @ W_o + b_o + x

Distribution: data-parallel over batch — 8 batch elements, one per NeuronCore.

v2 schedule: deep software pipeline. All scores for a query-half are issued
back-to-back (ScalarE exp chases them through a 3-deep PSUM pool into a
32-deep SBUF e-tile pool); ctx accumulation for both halves runs afterwards,
so the tensor queue never blocks on exp. Projections are ordered KT-first so
score matmuls flow without waiting on proj copies. Bias + residual are folded
host-side (xb = x + b_o). PSUM is time-multiplexed: {proj, scores} pools in
phase 1, {ctx, epilogue} pools in phase 2.

Softmax max-subtraction is skipped: scores/sqrt(U) are bounded (~±6) for any
well-scaled input, exp stays in f32/bf16 range.
"""

import numpy as np
import ml_dtypes

import concourse.bass as bass
import concourse.tile as tile
from concourse import bacc, mybir
from concourse.bass_utils import run_bass_kernel_spmd

N = 2048
D = 512
U = 128
NB = N // 128  # 16 query/key blocks
DC = D // 128  # 4 d-chunks
NS = N // 512  # 4 free-dim slices of 512
HQ = N // 2  # queries per half
INV_SQRT_U = 1.0 / np.sqrt(U)

F32 = mybir.dt.float32
BF16 = mybir.dt.bfloat16


def build_attention_nc():
    nc = bacc.Bacc("TRN2", target_bir_lowering=False, debug=False)

    xb_ext = nc.declare_dram_parameter("xb", [N, D], F32, isOutput=False)
    xT_ext = nc.declare_dram_parameter("xT", [D, N], BF16, isOutput=False)
    wq_ext = nc.declare_dram_parameter("wq", [128, D], BF16, isOutput=False)
    wk_ext = nc.declare_dram_parameter("wk", [128, D], BF16, isOutput=False)
    wv_ext = nc.declare_dram_parameter("wv", [128, D], BF16, isOutput=False)
    wo_ext = nc.declare_dram_parameter("wo", [U, D], BF16, isOutput=False)
    out_ext = nc.declare_dram_parameter("out", [N, D], F32, isOutput=True)

    with tile.TileContext(nc) as tc:
        _build_body(nc, tc, xb_ext, xT_ext, wq_ext, wk_ext, wv_ext, wo_ext, out_ext)
    nc.compile()
    return nc


def _build_body(nc, tc, xb_ext, xT_ext, wq_ext, wk_ext, wv_ext, wo_ext, out_ext):
    from contextlib import ExitStack

    with ExitStack() as ctx:
        const = ctx.enter_context(tc.tile_pool(name="const", bufs=1))

        # ---- loads: weights first (they gate the first matmuls) ----
        wq_sb = const.tile([128, D], BF16)  # [d-within-chunk, (c u)]
        wk_sb = const.tile([128, D], BF16)
        wv_sb = const.tile([128, D], BF16)
        wo_sb = const.tile([U, D], BF16)
        nc.scalar.dma_start(wk_sb[:], wk_ext.ap())
        nc.scalar.dma_start(wq_sb[:], wq_ext.ap())
        nc.gpsimd.dma_start(wv_sb[:], wv_ext.ap())
        nc.gpsimd.dma_start(wo_sb[:], wo_ext.ap())

        ones_sb = const.tile([128, 1], BF16)
        nc.vector.memset(ones_sb[:], 1.0)
        # force the exp activation table load while DMAs are in flight
        scratch = const.tile([128, 1], F32)
        nc.scalar.activation(
            scratch[:], ones_sb[:], mybir.ActivationFunctionType.Exp, scale=1.0
        )

        # x.T, d-chunk c at [:, c, :]; piecewise ns-major so the first KT/QT
        # projection can start after ~4 pieces instead of the full 2MB
        xT_sb = const.tile([128, DC, N], BF16)
        xT_r = xT_ext.ap().rearrange("(c p) n -> c p n", p=128)
        for ns in range(NS):
            for c in range(DC):
                nc.sync.dma_start(
                    xT_sb[:, c, ns * 512:(ns + 1) * 512],
                    xT_r[c][:, ns * 512:(ns + 1) * 512],
                )

        # xb = x + b_o precomputed on host; needed only for the epilogue
        xb_sb = const.tile([128, NB, D], F32)
        nc.gpsimd.dma_start(
            xb_sb[:], xb_ext.ap().rearrange("(nb p) d -> p nb d", p=128)
        )

        QT_sb = const.tile([U, N], BF16)
        KT_sb = const.tile([U, N], BF16)
        V_sb = const.tile([128, N], BF16)  # k-block kb at [:, kb*128:(kb+1)*128]
        ctxT_sb = const.tile([U, N], BF16)
        Esum = [const.tile([128, HQ], BF16, name=f"esum_{h}") for h in range(2)]
        r_sb = const.tile([128, NB], F32)

        def proj_slice(pool, w_sb, oT, ns):
            ps = pool.tile([128, 512], F32, tag="ps", name=f"pp_{oT.tensor.name}_{ns}")
            for c in range(DC):
                nc.tensor.matmul(
                    ps[:],
                    lhsT=w_sb[:, c * 128:(c + 1) * 128],
                    rhs=xT_sb[:, c, ns * 512:(ns + 1) * 512],
                    start=(c == 0),
                    stop=(c == DC - 1),
                )
            nc.vector.tensor_copy(oT[:, ns * 512:(ns + 1) * 512], ps[:])

        def make_v(pool, kb):
            ps = pool.tile([128, 512], F32, tag="ps", name=f"v_{kb}")
            for c in range(DC):
                nc.tensor.matmul(
                    ps[:, 0:128],
                    lhsT=xT_sb[:, c, kb * 128:(kb + 1) * 128],
                    rhs=wv_sb[:, c * 128:(c + 1) * 128],
                    start=(c == 0),
                    stop=(c == DC - 1),
                )
            nc.vector.tensor_copy(V_sb[:, kb * 128:(kb + 1) * 128], ps[:, 0:128])

        e_t = [None] * (2 * NB)  # e-tiles for both halves stay live

        # ---- phase 1: projections + all scores/exp (PSUM: pp 2 + sp 6 banks) ----
        ep = ctx.enter_context(tc.tile_pool(name="e_sb", bufs=32))
        with (
            tc.tile_pool(name="proj_ps", bufs=2, space="PSUM") as pp,
            tc.tile_pool(name="s_ps", bufs=3, space="PSUM") as sp,
        ):
            for ns in range(NS):
                proj_slice(pp, wk_sb, KT_sb, ns)
            proj_slice(pp, wq_sb, QT_sb, 0)
            proj_slice(pp, wq_sb, QT_sb, 1)

            def scores_block(h, kb):
                i = h * NB + kb
                q0 = h * HQ
                e_t[i] = ep.tile([128, HQ], BF16, tag="e", name=f"e_{h}_{kb}")
                s_ps = sp.tile([128, HQ], F32, tag="s", name=f"s_{h}_{kb}")
                for j in range(2):
                    nc.tensor.matmul(
                        s_ps[:, j * 512:(j + 1) * 512],
                        lhsT=KT_sb[:, kb * 128:(kb + 1) * 128],
                        rhs=QT_sb[:, q0 + j * 512:q0 + (j + 1) * 512],
                        start=True,
                        stop=True,
                    )
                nc.scalar.activation(
                    e_t[i][:],
                    s_ps[:],
                    mybir.ActivationFunctionType.Exp,
                    scale=INV_SQRT_U,
                )
                if kb == 0:
                    nc.vector.tensor_copy(Esum[h][:], e_t[i][:])
                else:
                    nc.vector.tensor_add(Esum[h][:], Esum[h][:], e_t[i][:])

            for kb in range(NB):
                scores_block(0, kb)

            proj_slice(pp, wq_sb, QT_sb, 2)
            proj_slice(pp, wq_sb, QT_sb, 3)
            for kb in range(NB):
                make_v(pp, kb)

            for kb in range(NB):
                scores_block(1, kb)

        # ---- phase 2: ctx accumulation + epilogue (PSUM: cp 4 + dp 3 banks) ----
        with (
            tc.tile_pool(name="ctx_ps", bufs=4, space="PSUM") as cp,
            tc.tile_pool(name="d_ps", bufs=3, space="PSUM") as dp,
            tc.tile_pool(name="o_sb", bufs=4) as op,
        ):
            def ctx_half(h):
                ctx_ps = [
                    cp.tile([U, 512], F32, tag="ctx", name=f"ctx_ps_{h}_{j}")
                    for j in range(2)
                ]
                return ctx_ps

            def ctx_mms(h, kb, ctx_ps):
                i = h * NB + kb
                for j in range(2):
                    nc.tensor.matmul(
                        ctx_ps[j][:],
                        lhsT=V_sb[:, kb * 128:(kb + 1) * 128],
                        rhs=e_t[i][:, j * 512:(j + 1) * 512],
                        start=(kb == 0),
                        stop=(kb == NB - 1),
                    )

            def rowsum(h):
                scol = dp.tile([128, 512], F32, tag="d", name=f"scol_{h}")
                for i in range(8):
                    nc.tensor.matmul(
                        scol[:, i:i + 1],
                        lhsT=Esum[h][:, i * 128:(i + 1) * 128],
                        rhs=ones_sb[:],
                        start=(i == 0),
                        stop=(i == 7),
                        skip_group_check=True,
                    )
                nc.vector.reciprocal(r_sb[:, h * 8:h * 8 + 8], scol[:, 0:8])

            def ctx_copy(h, j, ctx_ps):
                # PSUM ctx -> SBUF bf16 (DVE 16-bit-out runs at 2x)
                nc.vector.tensor_copy(
                    ctxT_sb[:, h * HQ + j * 512:h * HQ + (j + 1) * 512], ctx_ps[j][:]
                )

            def epilogue_qb(h, qb_local, store_eng):
                qb = h * 8 + qb_local
                y_ps = dp.tile([128, D], F32, tag="d", name=f"y_{qb}")
                nc.tensor.matmul(
                    y_ps[:],
                    lhsT=ctxT_sb[:, qb * 128:(qb + 1) * 128],
                    rhs=wo_sb[:],
                    start=True,
                    stop=True,
                )
                o_t = op.tile([128, D], F32, tag="o", name=f"o_{qb}")
                nc.vector.scalar_tensor_tensor(
                    o_t[:],
                    in0=y_ps[:],
                    scalar=r_sb[:, qb:qb + 1],
                    in1=xb_sb[:, qb, :],
                    op0=mybir.AluOpType.mult,
                    op1=mybir.AluOpType.add,
                )
                store_eng.dma_start(out_ext.ap()[qb * 128:(qb + 1) * 128, :], o_t[:])

            # ctx for half 0, straight through (exp long done)
            ctx0 = ctx_half(0)
            for kb in range(NB):
                ctx_mms(0, kb, ctx0)
            rowsum(0)
            ctx_copy(0, 0, ctx0)
            ctx_copy(0, 1, ctx0)

            # half-0 epilogue interleaved with half-1 ctx accumulation
            ctx1 = ctx_half(1)
            for qb_local in range(8):
                epilogue_qb(0, qb_local, nc.sync if qb_local % 2 == 0 else nc.scalar)
                ctx_mms(1, 2 * qb_local, ctx1)
                ctx_mms(1, 2 * qb_local + 1, ctx1)
                if qb_local == 0:
                    rowsum(1)

            ctx_copy(1, 0, ctx1)
            ctx_copy(1, 1, ctx1)
            for qb_local in range(8):
                epilogue_qb(1, qb_local, nc.sync if qb_local % 2 == 0 else nc.scalar)


_NC_CACHE = {}


def _get_nc():
    if "nc" not in _NC_CACHE:
        _NC_CACHE["nc"] = build_attention_nc()
    return _NC_CACHE["nc"]


def prep_in_maps(inputs, W_q, W_k, W_v, W_o, b_o):
    """Host-side sharding + layout prep. One batch element per core."""
    B = inputs.shape[0]
    bf = ml_dtypes.bfloat16

    def rearr_w(w):  # [D, U] -> [128, (c u)] with d = c*128 + p
        return np.ascontiguousarray(
            w.reshape(DC, 128, U).transpose(1, 0, 2).reshape(128, DC * U)
        ).astype(bf)

    wq_r = rearr_w(np.asarray(W_q))
    wk_r = rearr_w(np.asarray(W_k))
    wv_r = rearr_w(np.asarray(W_v))
    wo_r = np.ascontiguousarray(np.asarray(W_o)).astype(bf)
    bo = np.asarray(b_o, dtype=np.float32)

    in_maps = []
    for b in range(B):
        xf = np.asarray(inputs[b], dtype=np.float32)
        in_maps.append({
            "xb": np.ascontiguousarray(xf + bo),
            "xT": np.ascontiguousarray(xf.T).astype(bf),
            "wq": wq_r,
            "wk": wk_r,
            "wv": wv_r,
            "wo": wo_r,
        })
    return in_maps


def kernel(inputs, W_q, W_k, W_v, W_o, b_o):
    in_maps = prep_in_maps(inputs, W_q, W_k, W_v, W_o, b_o)
    nc = _get_nc()
    res = run_bass_kernel_spmd(nc, in_maps, core_ids=list(range(len(in_maps))))
    return np.stack([res.results[i]["out"] for i in range(len(in_maps))], axis=0)


# revision 10
# speedup vs baseline: 1.0983x; 1.0983x over previous
"""Trainium2 Bass kernel for single-head attention + output projection + residual.

Math per batch element b (N=2048, D=512, U=128):
    Q = x @ W_q; K = x @ W_k; V = x @ W_v
    S = Q @ K.T / sqrt(U); A = softmax(S, axis=-1)
    out = (A @ V) @ W_o + b_o + x

Distribution: data-parallel over batch — 8 batch elements, one per NeuronCore.

v2 schedule: deep software pipeline. All scores for a query-half are issued
back-to-back (ScalarE exp chases them through a 3-deep PSUM pool into a
32-deep SBUF e-tile pool); ctx accumulation for both halves runs afterwards,
so the tensor queue never blocks on exp. Projections are ordered KT-first so
score matmuls flow without waiting on proj copies. Bias + residual are folded
host-side (xb = x + b_o). PSUM is time-multiplexed: {proj, scores} pools in
phase 1, {ctx, epilogue} pools in phase 2.

Softmax max-subtraction is skipped: scores/sqrt(U) are bounded (~±6) for any
well-scaled input, exp stays in f32/bf16 range.
"""

import numpy as np
import ml_dtypes

import concourse.bass as bass
import concourse.tile as tile
from concourse import bacc, mybir
from concourse.bass_utils import run_bass_kernel_spmd

N = 2048
D = 512
U = 128
NB = N // 128  # 16 query/key blocks
DC = D // 128  # 4 d-chunks
NS = N // 512  # 4 free-dim slices of 512
HQ = N // 2  # queries per half
INV_SQRT_U = 1.0 / np.sqrt(U)

F32 = mybir.dt.float32
BF16 = mybir.dt.bfloat16


def build_attention_nc():
    nc = bacc.Bacc("TRN2", target_bir_lowering=False, debug=False)

    xb_ext = nc.declare_dram_parameter("xb", [N, D], F32, isOutput=False)
    xT_ext = nc.declare_dram_parameter("xT", [D, N], BF16, isOutput=False)
    wq_ext = nc.declare_dram_parameter("wq", [128, D], BF16, isOutput=False)
    wk_ext = nc.declare_dram_parameter("wk", [128, D], BF16, isOutput=False)
    wv_ext = nc.declare_dram_parameter("wv", [128, D], BF16, isOutput=False)
    wo_ext = nc.declare_dram_parameter("wo", [U, D], BF16, isOutput=False)
    out_ext = nc.declare_dram_parameter("out", [N, D], F32, isOutput=True)

    with tile.TileContext(nc) as tc:
        _build_body(nc, tc, xb_ext, xT_ext, wq_ext, wk_ext, wv_ext, wo_ext, out_ext)
    nc.compile()
    return nc


def _build_body(nc, tc, xb_ext, xT_ext, wq_ext, wk_ext, wv_ext, wo_ext, out_ext):
    from contextlib import ExitStack

    with ExitStack() as ctx:
        const = ctx.enter_context(tc.tile_pool(name="const", bufs=1))

        # ---- loads: weights first (they gate the first matmuls) ----
        wq_sb = const.tile([128, D], BF16)  # [d-within-chunk, (c u)]
        wk_sb = const.tile([128, D], BF16)
        wv_sb = const.tile([128, D], BF16)
        wo_sb = const.tile([U, D], BF16)
        nc.scalar.dma_start(wk_sb[:], wk_ext.ap())
        nc.scalar.dma_start(wq_sb[:], wq_ext.ap())
        nc.gpsimd.dma_start(wv_sb[:], wv_ext.ap())
        nc.gpsimd.dma_start(wo_sb[:], wo_ext.ap())

        ones_sb = const.tile([128, 1], BF16)
        nc.vector.memset(ones_sb[:], 1.0)
        # force the exp activation table load while DMAs are in flight
        scratch = const.tile([128, 1], F32)
        nc.scalar.activation(
            scratch[:], ones_sb[:], mybir.ActivationFunctionType.Exp, scale=1.0
        )

        # x.T, d-chunk c at [:, c, :]; piecewise ns-major so the first KT/QT
        # projection can start after ~4 pieces instead of the full 2MB
        xT_sb = const.tile([128, DC, N], BF16)
        xT_r = xT_ext.ap().rearrange("(c p) n -> c p n", p=128)
        for ns in range(NS):
            for c in range(DC):
                nc.sync.dma_start(
                    xT_sb[:, c, ns * 512:(ns + 1) * 512],
                    xT_r[c][:, ns * 512:(ns + 1) * 512],
                )

        # xb = x + b_o precomputed on host; needed only for the epilogue.
        # Issued on the same queue AFTER all xT pieces so the 4MB load cannot
        # steal HBM bandwidth from the projection-critical xT.
        xb_sb = const.tile([128, NB, D], F32)
        nc.sync.dma_start(
            xb_sb[:], xb_ext.ap().rearrange("(nb p) d -> p nb d", p=128)
        )

        QT_sb = const.tile([U, N], BF16)
        KT_sb = const.tile([U, N], BF16)
        V_sb = const.tile([128, N], BF16)  # k-block kb at [:, kb*128:(kb+1)*128]
        ctxT_sb = const.tile([U, N], BF16)
        Esum = [const.tile([128, HQ], BF16, name=f"esum_{h}") for h in range(2)]
        r_sb = const.tile([128, NB], F32)

        def proj_slice(pool, w_sb, oT, ns):
            ps = pool.tile([128, 512], F32, tag="ps", name=f"pp_{oT.tensor.name}_{ns}")
            for c in range(DC):
                nc.tensor.matmul(
                    ps[:],
                    lhsT=w_sb[:, c * 128:(c + 1) * 128],
                    rhs=xT_sb[:, c, ns * 512:(ns + 1) * 512],
                    start=(c == 0),
                    stop=(c == DC - 1),
                )
            nc.vector.tensor_copy(oT[:, ns * 512:(ns + 1) * 512], ps[:])

        def make_v(pool, kb):
            ps = pool.tile([128, 512], F32, tag="ps", name=f"v_{kb}")
            for c in range(DC):
                nc.tensor.matmul(
                    ps[:, 0:128],
                    lhsT=xT_sb[:, c, kb * 128:(kb + 1) * 128],
                    rhs=wv_sb[:, c * 128:(c + 1) * 128],
                    start=(c == 0),
                    stop=(c == DC - 1),
                )
            nc.vector.tensor_copy(V_sb[:, kb * 128:(kb + 1) * 128], ps[:, 0:128])

        e_t = [None] * (2 * NB)  # e-tiles for both halves stay live

        # ---- phase 1: projections + all scores/exp (PSUM: pp 2 + sp 6 banks) ----
        ep = ctx.enter_context(tc.tile_pool(name="e_sb", bufs=32))
        with (
            tc.tile_pool(name="proj_ps", bufs=2, space="PSUM") as pp,
            tc.tile_pool(name="s_ps", bufs=3, space="PSUM") as sp,
        ):
            for ns in range(NS):
                proj_slice(pp, wk_sb, KT_sb, ns)
            proj_slice(pp, wq_sb, QT_sb, 0)
            proj_slice(pp, wq_sb, QT_sb, 1)

            def scores_block(h, kb):
                i = h * NB + kb
                q0 = h * HQ
                e_t[i] = ep.tile([128, HQ], BF16, tag="e", name=f"e_{h}_{kb}")
                s_ps = sp.tile([128, HQ], F32, tag="s", name=f"s_{h}_{kb}")
                for j in range(2):
                    nc.tensor.matmul(
                        s_ps[:, j * 512:(j + 1) * 512],
                        lhsT=KT_sb[:, kb * 128:(kb + 1) * 128],
                        rhs=QT_sb[:, q0 + j * 512:q0 + (j + 1) * 512],
                        start=True,
                        stop=True,
                    )
                nc.scalar.activation(
                    e_t[i][:],
                    s_ps[:],
                    mybir.ActivationFunctionType.Exp,
                    scale=INV_SQRT_U,
                )
                if kb == 0:
                    nc.vector.tensor_copy(Esum[h][:], e_t[i][:])
                else:
                    nc.vector.tensor_add(Esum[h][:], Esum[h][:], e_t[i][:])

            for kb in range(NB):
                scores_block(0, kb)

            proj_slice(pp, wq_sb, QT_sb, 2)
            proj_slice(pp, wq_sb, QT_sb, 3)
            for kb in range(NB):
                make_v(pp, kb)

            for kb in range(NB):
                scores_block(1, kb)

        # ---- phase 2: ctx accumulation + epilogue (PSUM: cp 4 + dp 3 banks) ----
        with (
            tc.tile_pool(name="ctx_ps", bufs=4, space="PSUM") as cp,
            tc.tile_pool(name="d_ps", bufs=3, space="PSUM") as dp,
            tc.tile_pool(name="o_sb", bufs=4) as op,
        ):
            def ctx_half(h):
                ctx_ps = [
                    cp.tile([U, 512], F32, tag="ctx", name=f"ctx_ps_{h}_{j}")
                    for j in range(2)
                ]
                return ctx_ps

            def ctx_mms(h, kb, ctx_ps):
                i = h * NB + kb
                for j in range(2):
                    nc.tensor.matmul(
                        ctx_ps[j][:],
                        lhsT=V_sb[:, kb * 128:(kb + 1) * 128],
                        rhs=e_t[i][:, j * 512:(j + 1) * 512],
                        start=(kb == 0),
                        stop=(kb == NB - 1),
                    )

            def rowsum(h):
                scol = dp.tile([128, 512], F32, tag="d", name=f"scol_{h}")
                for i in range(8):
                    nc.tensor.matmul(
                        scol[:, i:i + 1],
                        lhsT=Esum[h][:, i * 128:(i + 1) * 128],
                        rhs=ones_sb[:],
                        start=(i == 0),
                        stop=(i == 7),
                        skip_group_check=True,
                    )
                nc.vector.reciprocal(r_sb[:, h * 8:h * 8 + 8], scol[:, 0:8])

            def ctx_copy(h, j, ctx_ps):
                # PSUM ctx -> SBUF bf16 (DVE 16-bit-out runs at 2x)
                nc.vector.tensor_copy(
                    ctxT_sb[:, h * HQ + j * 512:h * HQ + (j + 1) * 512], ctx_ps[j][:]
                )

            def epilogue_qb(h, qb_local, store_eng):
                qb = h * 8 + qb_local
                y_ps = dp.tile([128, D], F32, tag="d", name=f"y_{qb}")
                nc.tensor.matmul(
                    y_ps[:],
                    lhsT=ctxT_sb[:, qb * 128:(qb + 1) * 128],
                    rhs=wo_sb[:],
                    start=True,
                    stop=True,
                )
                o_t = op.tile([128, D], F32, tag="o", name=f"o_{qb}")
                nc.vector.scalar_tensor_tensor(
                    o_t[:],
                    in0=y_ps[:],
                    scalar=r_sb[:, qb:qb + 1],
                    in1=xb_sb[:, qb, :],
                    op0=mybir.AluOpType.mult,
                    op1=mybir.AluOpType.add,
                )
                store_eng.dma_start(out_ext.ap()[qb * 128:(qb + 1) * 128, :], o_t[:])

            # ctx for half 0, straight through (exp long done)
            ctx0 = ctx_half(0)
            for kb in range(NB):
                ctx_mms(0, kb, ctx0)
            rowsum(0)
            ctx_copy(0, 0, ctx0)
            ctx_copy(0, 1, ctx0)

            # half-0 epilogue interleaved with half-1 ctx accumulation
            ctx1 = ctx_half(1)
            for qb_local in range(8):
                epilogue_qb(0, qb_local, nc.sync if qb_local % 2 == 0 else nc.scalar)
                ctx_mms(1, 2 * qb_local, ctx1)
                ctx_mms(1, 2 * qb_local + 1, ctx1)
                if qb_local == 0:
                    rowsum(1)

            ctx_copy(1, 0, ctx1)
            ctx_copy(1, 1, ctx1)
            for qb_local in range(8):
                epilogue_qb(1, qb_local, nc.sync if qb_local % 2 == 0 else nc.scalar)


_NC_CACHE = {}


def _get_nc():
    if "nc" not in _NC_CACHE:
        _NC_CACHE["nc"] = build_attention_nc()
    return _NC_CACHE["nc"]


def prep_in_maps(inputs, W_q, W_k, W_v, W_o, b_o):
    """Host-side sharding + layout prep. One batch element per core."""
    B = inputs.shape[0]
    bf = ml_dtypes.bfloat16

    def rearr_w(w):  # [D, U] -> [128, (c u)] with d = c*128 + p
        return np.ascontiguousarray(
            w.reshape(DC, 128, U).transpose(1, 0, 2).reshape(128, DC * U)
        ).astype(bf)

    wq_r = rearr_w(np.asarray(W_q))
    wk_r = rearr_w(np.asarray(W_k))
    wv_r = rearr_w(np.asarray(W_v))
    wo_r = np.ascontiguousarray(np.asarray(W_o)).astype(bf)
    bo = np.asarray(b_o, dtype=np.float32)

    in_maps = []
    for b in range(B):
        xf = np.asarray(inputs[b], dtype=np.float32)
        in_maps.append({
            "xb": np.ascontiguousarray(xf + bo),
            "xT": np.ascontiguousarray(xf.T).astype(bf),
            "wq": wq_r,
            "wk": wk_r,
            "wv": wv_r,
            "wo": wo_r,
        })
    return in_maps


def kernel(inputs, W_q, W_k, W_v, W_o, b_o):
    in_maps = prep_in_maps(inputs, W_q, W_k, W_v, W_o, b_o)
    nc = _get_nc()
    res = run_bass_kernel_spmd(nc, in_maps, core_ids=list(range(len(in_maps))))
    return np.stack([res.results[i]["out"] for i in range(len(in_maps))], axis=0)
